# revision 11
# baseline (speedup 1.0000x reference)
"""AttentionBlock (GroupNorm + single-head-dim-64 x4-head attention + proj,
residual) on 8 Trainium2 NeuronCores.

Sharding: data-parallel over batch (B=4) x query-position halves (2 per
batch) -> 8 shards, one per core. Each core:
  - GroupNorm(32) of its batch's x [256, 4096] (duplicated across the 2
    cores of a batch -- cheap),
  - qkv projection (k, v for all 4 heads over all 4096 positions; q only for
    the core's 2048 query positions),
  - flash-style attention: per (head, 512-query block), stream 128-wide key
    chunks: scores^T = k_chunk.T @ q -> exp on ScalarE -> accumulate
    exp(w).T @ [v | 1] on TensorE (the appended ones-column yields the
    softmax denominator for free),
  - divide by denominator, output projection, +bias +residual, DMA out.
Host gathers the 8 [256, 2048] outputs into [4, 256, 64, 64].

Query-half selection is done by ROTATING x's spatial axis on the host for
odd cores (softmax/groupnorm sums are permutation-invariant), so a single
SPMD program serves all 8 cores.
"""

import numpy as np

import concourse.bass as bass
import concourse.tile as tile
from concourse import bacc, mybir
from concourse.bass import AP

F32 = mybir.dt.float32
AF = mybir.ActivationFunctionType
ALU = mybir.AluOpType

B, C, HH, WW = 4, 256, 64, 64
N = HH * WW            # 4096 spatial positions
NH = 4                 # heads
HD = C // NH           # 64 head dim
NG = 32                # groupnorm groups
EPS = 1e-5
NCORES = 8
THALF = N // 2         # 2048 query positions per core
TB = 512               # query-block width
NTB = THALF // TB      # 4
SCW = 128              # key-chunk width
NSC = N // SCW         # 32
SCALE = float(HD) ** -0.25

_PROGRAMS = {}         # (qkv_bias, proj_bias) -> compiled Bacc
_RUNNERS = {}          # id(nc) -> callable(in_maps) -> list of out dicts


def _build_program(with_qkv_bias: bool, with_proj_bias: bool, dbg: bool = False):
    nc = bacc.Bacc(
        "TRN2", target_bir_lowering=False, debug=False, num_devices=NCORES
    )
    if dbg:
        dxn = nc.declare_dram_parameter("dxn", [128, 2, N], F32, isOutput=True)
        dk = nc.declare_dram_parameter("dk", [128, 2, N], F32, isOutput=True)
        dq = nc.declare_dram_parameter("dq", [128, 2, THALF], F32, isOutput=True)
        dvt = nc.declare_dram_parameter(
            "dvt", [128, NSC, NH, HD + 1], F32, isOutput=True
        )
        dqk = nc.declare_dram_parameter("dqk", [128, 2 * TB], F32, isOutput=True)
        dacc = nc.declare_dram_parameter("dacc", [HD + 1, TB], F32, isOutput=True)
        dpc = nc.declare_dram_parameter("dpc", [128, 2, 2], F32, isOutput=True)
        dht = nc.declare_dram_parameter("dht", [128, 2, TB], F32, isOutput=True)
        drec = nc.declare_dram_parameter("drec", [1, TB], F32, isOutput=True)
        dbc = nc.declare_dram_parameter("dbc", [64, TB], F32, isOutput=True)

    xb = nc.declare_dram_parameter("xb", [C, N], F32, isOutput=False)
    # columns: [0:256] q^T (scaled), [256:512] k^T (scaled), [512:768] v^T,
    # all in head-major order (col 64h+j of a block = qkv_w row 192h + off + j)
    wt = nc.declare_dram_parameter("wt", [C, 3 * C], F32, isOutput=False)
    pt = nc.declare_dram_parameter("pt", [C, C], F32, isOutput=False)  # proj_w.T
    gmat = nc.declare_dram_parameter("gmat", [128, 16], F32, isOutput=False)
    gmatt = nc.declare_dram_parameter("gmatt", [16, 128], F32, isOutput=False)
    gnw = nc.declare_dram_parameter("gnw", [C, 1], F32, isOutput=False)
    gnb = nc.declare_dram_parameter("gnb", [C, 1], F32, isOutput=False)
    if with_qkv_bias:
        bq = nc.declare_dram_parameter("bq", [C, 1], F32, isOutput=False)
        bk = nc.declare_dram_parameter("bk", [C, 1], F32, isOutput=False)
        bv = nc.declare_dram_parameter("bv", [1, C], F32, isOutput=False)
    if with_proj_bias:
        pb = nc.declare_dram_parameter("pb", [C, 1], F32, isOutput=False)
    yout = nc.declare_dram_parameter("y", [C, THALF], F32, isOutput=True)

    with tile.TileContext(nc) as tc:
        with tc.tile_pool(name="persist", bufs=1) as PP:
            x0 = PP.tile([128, N], F32)
            x1 = PP.tile([128, N], F32)
            nc.sync.dma_start(x0[:], xb[0:128, :])
            nc.sync.dma_start(x1[:], xb[128:256, :])
            xc = [x0, x1]

            wts = PP.tile([128, 2, 3 * C], F32)
            nc.sync.dma_start(wts[:], wt.rearrange("(k p) m -> p k m", k=2))
            pts = PP.tile([128, 2, C], F32)
            nc.sync.dma_start(pts[:], pt.rearrange("(k p) m -> p k m", k=2))
            gm = PP.tile([128, 16], F32)
            nc.sync.dma_start(gm[:], gmat[:])
            gmt = PP.tile([16, 128], F32)
            nc.sync.dma_start(gmt[:], gmatt[:])
            gns = PP.tile([128, 2, 2], F32)
            nc.sync.dma_start(gns[:, :, 0:1], gnw.rearrange("(k p) o -> p k o", k=2))
            nc.sync.dma_start(gns[:, :, 1:2], gnb.rearrange("(k p) o -> p k o", k=2))
            if with_qkv_bias:
                bqs = PP.tile([128, 2, 1], F32)
                nc.sync.dma_start(bqs[:], bq.rearrange("(k p) o -> p k o", k=2))
                bks = PP.tile([128, 2, 1], F32)
                nc.sync.dma_start(bks[:], bk.rearrange("(k p) o -> p k o", k=2))
                bvb = PP.tile([128, NH, HD], F32)
                bv_ap = AP(tensor=bv.tensor, offset=bv.offset,
                           ap=[[0, 128], [HD, NH], [1, HD]])
                nc.sync.dma_start(bvb[:], bv_ap)
            if with_proj_bias:
                pbs = PP.tile([128, 2, 1], F32)
                nc.sync.dma_start(pbs[:], pb.rearrange("(k p) o -> p k o", k=2))

            k_sb = PP.tile([128, 2, N], F32)
            q_sb = PP.tile([128, 2, THALF], F32)
            vTp = PP.tile([128, NSC, NH, HD + 1], F32)
            nc.vector.memset(vTp[:, :, :, HD:HD + 1], 1.0)
            pcsb = PP.tile([128, 2, 2], F32)  # per-channel (scale, bias)
            eps16 = PP.tile([16, 1], F32)
            nc.vector.memset(eps16[:], EPS)

            # ---------------- GroupNorm statistics ----------------
            with (
                tc.tile_pool(name="gnp", bufs=2) as GP,
                tc.tile_pool(name="psg", bufs=1, space=bass.MemorySpace.PSUM) as PSG,
            ):
                psg = PSG.tile([16, 6], F32)
                for ci in range(2):
                    st = GP.tile([128, 8, 6], F32, tag="st")
                    for j in range(8):
                        nc.vector.bn_stats(
                            st[:, j, :], xc[ci][:, j * 512:(j + 1) * 512]
                        )
                    mv = GP.tile([128, 2], F32, tag="mv")
                    nc.vector.bn_aggr(mv[:], st[:])
                    s3 = GP.tile([128, 3], F32, tag="s3")
                    nc.vector.tensor_copy(s3[:, 0:2], mv[:])
                    nc.vector.tensor_mul(s3[:, 2:3], mv[:, 0:1], mv[:, 0:1])
                    # per-group sums over the 8 channels of each group
                    nc.tensor.matmul(
                        psg[:, 3 * ci:3 * ci + 3], gm[:], s3[:],
                        start=(ci == 0), stop=(ci == 1),
                    )
                gsb = GP.tile([16, 4], F32, tag="gsb")  # (rstd, mu*rstd) x2
                psgs = GP.tile([16, 6], F32, tag="psgs")
                nc.vector.tensor_copy(psgs[:], psg[:])
                for ci in range(2):
                    s1 = psgs[:, 3 * ci + 0:3 * ci + 1]
                    s2 = psgs[:, 3 * ci + 1:3 * ci + 2]
                    s3c = psgs[:, 3 * ci + 2:3 * ci + 3]
                    t4 = GP.tile([16, 4], F32, tag="t4")
                    nc.vector.tensor_scalar_mul(t4[:, 0:1], s1, 0.125)   # mu_g
                    nc.vector.tensor_add(t4[:, 1:2], s2, s3c)
                    nc.vector.tensor_scalar_mul(t4[:, 1:2], t4[:, 1:2], 0.125)
                    nc.vector.tensor_mul(t4[:, 2:3], t4[:, 0:1], t4[:, 0:1])
                    nc.vector.tensor_sub(t4[:, 1:2], t4[:, 1:2], t4[:, 2:3])
                    # sd = sqrt(var + eps)
                    nc.scalar.activation(
                        t4[:, 3:4], t4[:, 1:2], AF.Sqrt, bias=eps16[:], scale=1.0
                    )
                    nc.vector.reciprocal(gsb[:, 2 * ci:2 * ci + 1], t4[:, 3:4])
                    nc.vector.tensor_mul(
                        gsb[:, 2 * ci + 1:2 * ci + 2],
                        t4[:, 0:1], gsb[:, 2 * ci:2 * ci + 1],
                    )
                # group -> channel broadcast: G.T [16,128] expansion matmul
                g2cp = PSG.tile([128, 4], F32, tag="g2cp")
                nc.tensor.matmul(g2cp[:], gmt[:], gsb[:], start=True, stop=True)
                g2c = GP.tile([128, 4], F32, tag="g2c")
                nc.vector.tensor_copy(g2c[:], g2cp[:])
                for ci in range(2):
                    # scale_c = rstd*gn_w ; bias_c = gn_b - mu*rstd*gn_w
                    nc.vector.tensor_mul(
                        pcsb[:, ci, 0:1], g2c[:, 2 * ci:2 * ci + 1],
                        gns[:, ci, 0:1]
                    )
                    nc.vector.tensor_mul(
                        pcsb[:, ci, 1:2], g2c[:, 2 * ci + 1:2 * ci + 2],
                        gns[:, ci, 0:1]
                    )
                    nc.vector.tensor_sub(
                        pcsb[:, ci, 1:2], gns[:, ci, 1:2], pcsb[:, ci, 1:2]
                    )

            # ---------------- normalize + qkv projections ----------------
            with (
                tc.tile_pool(name="xnp", bufs=1) as XP,
                tc.tile_pool(name="psb", bufs=1, space=bass.MemorySpace.PSUM) as PSB,
            ):
                xn0 = XP.tile([128, N], F32)
                xn1 = XP.tile([128, N], F32)
                xn = [xn0, xn1]
                for ci in range(2):
                    nc.vector.tensor_scalar(
                        xn[ci][:], xc[ci][:],
                        pcsb[:, ci, 0:1], pcsb[:, ci, 1:2],
                        op0=ALU.mult, op1=ALU.add,
                    )
                if dbg:
                    nc.sync.dma_start(dxn[:, 0, :], xn0[:])
                    nc.sync.dma_start(dxn[:, 1, :], xn1[:])
                    nc.sync.dma_start(dpc[:], pcsb[:])
                # k (full n) and q (first THALF columns)
                for m in range(2):
                    for j in range(8):
                        ps = PSB.tile([128, 512], F32, tag="kq", bufs=3)
                        for kc in range(2):
                            nc.tensor.matmul(
                                ps[:],
                                wts[:, kc, 256 + 128 * m:256 + 128 * m + 128],
                                xn[kc][:, 512 * j:512 * j + 512],
                                start=(kc == 0), stop=(kc == 1),
                            )
                        if with_qkv_bias:
                            nc.scalar.activation(
                                k_sb[:, m, 512 * j:512 * j + 512], ps[:],
                                AF.Identity, bias=bks[:, m, :], scale=1.0,
                            )
                        else:
                            nc.scalar.copy(
                                k_sb[:, m, 512 * j:512 * j + 512], ps[:]
                            )
                    for j in range(4):
                        ps = PSB.tile([128, 512], F32, tag="kq", bufs=3)
                        for kc in range(2):
                            nc.tensor.matmul(
                                ps[:],
                                wts[:, kc, 128 * m:128 * m + 128],
                                xn[kc][:, 512 * j:512 * j + 512],
                                start=(kc == 0), stop=(kc == 1),
                            )
                        if with_qkv_bias:
                            nc.scalar.activation(
                                q_sb[:, m, 512 * j:512 * j + 512], ps[:],
                                AF.Identity, bias=bqs[:, m, :], scale=1.0,
                            )
                        else:
                            nc.scalar.copy(
                                q_sb[:, m, 512 * j:512 * j + 512], ps[:]
                            )
                # v^T directly: vT[s, c] = sum_ch xn[ch, s] * Wv^T[ch, c]
                for sc in range(NSC):
                    ps = PSB.tile([128, 256], F32, tag="v", bufs=3)
                    for kc in range(2):
                        nc.tensor.matmul(
                            ps[:],
                            xn[kc][:, SCW * sc:SCW * sc + SCW],
                            wts[:, kc, 512:768],
                            start=(kc == 0), stop=(kc == 1),
                        )
                    psr = ps[:].rearrange("p (h d) -> p h d", h=NH)
                    if with_qkv_bias:
                        nc.vector.tensor_add(vTp[:, sc, :, 0:HD], psr, bvb[:])
                    else:
                        nc.vector.tensor_copy(vTp[:, sc, :, 0:HD], psr)

            # ---------------- attention + output projection ----------------
            with (
                tc.tile_pool(name="att_sb", bufs=1) as AS,
                tc.tile_pool(name="att_dr", bufs=1,
                             space=bass.MemorySpace.DRAM) as DP,
                tc.tile_pool(name="ps_qk", bufs=1, space=bass.MemorySpace.PSUM) as PQK,
                tc.tile_pool(name="ps_acc", bufs=1, space=bass.MemorySpace.PSUM) as PAC,
                tc.tile_pool(name="ps_y", bufs=1, space=bass.MemorySpace.PSUM) as PY,
            ):
                if dbg:
                    nc.sync.dma_start(dk[:], k_sb[:])
                    nc.sync.dma_start(dq[:], q_sb[:])
                    nc.sync.dma_start(dvt[:], vTp[:])
                for tb in range(NTB):
                    ht = AS.tile([128, 2, TB], F32, tag="ht", bufs=2)
                    for hp in range(2):
                        a0 = PAC.tile([HD + 1, TB], F32, tag="a0", bufs=1)
                        a1 = PAC.tile([HD + 1, TB], F32, tag="a1", bufs=1)
                        accs = [a0, a1]
                        for sc in range(NSC):
                            qk = PQK.tile([128, 2 * TB], F32, tag="qk", bufs=2)
                            for li in range(2):
                                nc.tensor.matmul(
                                    qk[:, TB * li:TB * li + TB],
                                    k_sb[64 * li:64 * li + 64, hp,
                                         SCW * sc:SCW * sc + SCW],
                                    q_sb[64 * li:64 * li + 64, hp,
                                         TB * tb:TB * tb + TB],
                                    start=True, stop=True,
                                )
                            ew = AS.tile([128, 2 * TB], F32, tag="ew", bufs=3)
                            if dbg and tb == 0 and hp == 0 and sc == 0:
                                dqt = AS.tile([128, 2 * TB], F32, tag="dqt")
                                nc.vector.tensor_copy(dqt[:], qk[:])
                                nc.sync.dma_start(dqk[:], dqt[:])
                            nc.scalar.activation(ew[:], qk[:], AF.Exp)
                            for li in range(2):
                                nc.tensor.matmul(
                                    accs[li][:],
                                    vTp[:, sc, 2 * hp + li, :],
                                    ew[:, TB * li:TB * li + TB],
                                    start=(sc == 0), stop=(sc == NSC - 1),
                                )
                        if dbg and tb == 0 and hp == 0:
                            dat = AS.tile([HD + 1, TB], F32, tag="dat")
                            nc.vector.tensor_copy(dat[:], a0[:])
                            nc.sync.dma_start(dacc[:], dat[:])
                        for li in range(2):
                            acc = accs[li]
                            # reciprocal of the denominator row, partition-
                            # aligned at partition 64 (DVE lanes cannot move
                            # data across partitions)
                            rec = AS.tile([HD + 1, TB], F32, tag="rec", bufs=2)
                            nc.vector.reciprocal(
                                rec[HD:HD + 1, :], acc[HD:HD + 1, :]
                            )
                            # broadcast across partitions via DRAM bounce
                            rscr = DP.tile([1, TB], F32, tag="rscr", bufs=2)
                            nc.sync.dma_start(rscr[:], rec[HD:HD + 1, :])
                            bc = AS.tile([64, TB], F32, tag="bc", bufs=2)
                            rs = rscr[:]
                            nc.sync.dma_start(
                                bc[:],
                                AP(tensor=rs.tensor, offset=rs.offset,
                                   ap=[[0, 64], [1, TB]]),
                            )
                            if dbg and tb == 0 and hp == 0 and li == 0:
                                nc.sync.dma_start(drec[:], rec[HD:HD + 1, :])
                                nc.sync.dma_start(dbc[:], bc[:])
                            nc.vector.tensor_mul(
                                ht[64 * li:64 * li + 64, hp, :],
                                acc[0:HD, :], bc[:],
                            )
                    if dbg and tb == 0:
                        nc.sync.dma_start(dht[:], ht[:])
                    for m in range(2):
                        yp = PY.tile([128, TB], F32, tag="yp", bufs=2)
                        for kc in range(2):
                            nc.tensor.matmul(
                                yp[:],
                                pts[:, kc, 128 * m:128 * m + 128],
                                ht[:, kc, :],
                                start=(kc == 0), stop=(kc == 1),
                            )
                        yo = AS.tile([128, TB], F32, tag="yo", bufs=2)
                        nc.vector.tensor_add(
                            yo[:], yp[:], xc[m][:, TB * tb:TB * tb + TB]
                        )
                        if with_proj_bias:
                            nc.vector.tensor_scalar_add(
                                yo[:], yo[:], pbs[:, m, :]
                            )
                        nc.sync.dma_start(
                            yout[128 * m:128 * m + 128, TB * tb:TB * tb + TB],
                            yo[:],
                        )

    nc.compile()
    return nc


def get_program(with_qkv_bias: bool, with_proj_bias: bool):
    key = (with_qkv_bias, with_proj_bias)
    if key not in _PROGRAMS:
        _PROGRAMS[key] = _build_program(*key)
    return _PROGRAMS[key]


def _make_runner(nc):
    """Cached jit-compiled SPMD runner (mirrors bass2jax.run_bass_via_pjrt's
    multi-core branch but reusable across calls)."""
    import jax
    from jax.experimental.shard_map import shard_map
    from jax.sharding import Mesh, PartitionSpec
    from concourse import bass2jax

    bass2jax.install_neuronx_cc_hook()

    in_names, out_names, out_avals, zero_outs = [], [], [], []
    for alloc in nc.m.functions[0].allocations:
        if not isinstance(alloc, mybir.MemoryLocationSet):
            continue
        name = alloc.memorylocations[0].name
        if alloc.kind == "ExternalInput":
            in_names.append(name)
        elif alloc.kind == "ExternalOutput":
            out_names.append(name)
            shape = tuple(alloc.tensor_shape)
            dtype = mybir.dt.np(alloc.dtype)
            out_avals.append(jax.core.ShapedArray(shape, dtype))
            zero_outs.append(np.zeros(shape, dtype))
    n_params = len(in_names)
    n_outs = len(out_avals)
    all_names = in_names + out_names
    donate = tuple(range(n_params, n_params + n_outs))

    def _body(*args):
        outs = bass2jax._bass_exec_p.bind(
            *args,
            out_avals=tuple(out_avals),
            in_names=tuple(all_names),
            out_names=tuple(out_names),
            lowering_input_output_aliases=(),
            sim_require_finite=True,
            sim_require_nnan=True,
            nc=nc,
        )
        return tuple(outs)

    devices = jax.devices()[:NCORES]
    mesh = Mesh(np.asarray(devices), ("core",))
    in_specs = (PartitionSpec("core"),) * (n_params + n_outs)
    out_specs = (PartitionSpec("core"),) * n_outs
    sharded = jax.jit(
        shard_map(_body, mesh=mesh, in_specs=in_specs, out_specs=out_specs,
                  check_rep=False),
        donate_argnums=donate,
        keep_unused=True,
    )

    pid_name = nc.partition_id_tensor.name if nc.partition_id_tensor else None

    def run(in_maps):
        concat_in = [
            np.concatenate(
                [
                    np.asarray(m[name])
                    if name in m
                    else np.array([[c]], dtype=np.uint32)
                    for c, m in enumerate(in_maps)
                ],
                axis=0,
            )
            if name == pid_name
            else np.concatenate([np.asarray(m[name]) for m in in_maps], axis=0)
            for name in in_names
        ]
        concat_zero = [
            np.zeros((NCORES * z.shape[0], *z.shape[1:]), z.dtype)
            for z in zero_outs
        ]
        out_arrs = sharded(*concat_in, *concat_zero)
        return [
            {
                name: np.asarray(out_arrs[i]).reshape(
                    NCORES, *out_avals[i].shape
                )[c]
                for i, name in enumerate(out_names)
            }
            for c in range(NCORES)
        ]

    run.in_names = in_names
    run.out_names = out_names
    return run


def get_runner(nc):
    key = id(nc)
    if key not in _RUNNERS:
        _RUNNERS[key] = _make_runner(nc)
    return _RUNNERS[key]


def prep_host_inputs(x, gn_w, gn_b, qkv_w, qkv_b, proj_w, proj_b):
    """Host-side weight layout transforms + per-core input maps."""
    x = np.asarray(x, np.float32)
    gn_w = np.asarray(gn_w, np.float32)
    gn_b = np.asarray(gn_b, np.float32)
    qkv_w = np.asarray(qkv_w, np.float32)
    qkv_b = np.asarray(qkv_b, np.float32)
    proj_w = np.asarray(proj_w, np.float32)
    proj_b = np.asarray(proj_b, np.float32)

    # head-major row indices into qkv_w for q / k / v
    hidx = np.arange(NH)[:, None] * (3 * HD) + np.arange(HD)[None, :]
    qrows = hidx.reshape(-1)
    krows = (hidx + HD).reshape(-1)
    vrows = (hidx + 2 * HD).reshape(-1)
    wt = np.empty((C, 3 * C), np.float32)
    wt[:, 0:C] = qkv_w[qrows].T * SCALE
    wt[:, C:2 * C] = qkv_w[krows].T * SCALE
    wt[:, 2 * C:3 * C] = qkv_w[vrows].T
    pt = np.ascontiguousarray(proj_w.T)

    gmat = np.zeros((128, 16), np.float32)
    gmat[np.arange(128), np.arange(128) // 8] = 1.0
    gmatt = np.ascontiguousarray(gmat.T)

    with_qkv_bias = bool(np.any(qkv_b))
    with_proj_bias = bool(np.any(proj_b))

    shared = dict(
        wt=wt, pt=pt, gmat=gmat, gmatt=gmatt,
        gnw=gn_w.reshape(C, 1).astype(np.float32),
        gnb=gn_b.reshape(C, 1).astype(np.float32),
    )
    if with_qkv_bias:
        shared["bq"] = (qkv_b[qrows] * SCALE).reshape(C, 1).astype(np.float32)
        shared["bk"] = (qkv_b[krows] * SCALE).reshape(C, 1).astype(np.float32)
        shared["bv"] = qkv_b[vrows].reshape(1, C).astype(np.float32)
    if with_proj_bias:
        shared["pb"] = proj_b.reshape(C, 1).astype(np.float32)

    xf = x.reshape(B, C, N)
    in_maps = []
    for core in range(NCORES):
        b, half = core // 2, core % 2
        if half == 0:
            xbc = np.ascontiguousarray(xf[b])
        else:
            xbc = np.ascontiguousarray(
                np.concatenate([xf[b][:, THALF:], xf[b][:, :THALF]], axis=1)
            )
        m = dict(shared)
        m["xb"] = xbc
        in_maps.append(m)
    return in_maps, with_qkv_bias, with_proj_bias


def assemble_output(results, x):
    out = np.empty((B, C, N), np.float32)
    for core in range(NCORES):
        b, half = core // 2, core % 2
        out[b][:, half * THALF:(half + 1) * THALF] = results[core]["y"]
    return out.reshape(B, C, HH, WW)


def kernel(x, gn_w, gn_b, qkv_w, qkv_b, proj_w, proj_b):
    in_maps, qb, pbf = prep_host_inputs(
        x, gn_w, gn_b, qkv_w, qkv_b, proj_w, proj_b
    )
    nc = get_program(qb, pbf)
    run = get_runner(nc)
    results = run(in_maps)
    return assemble_output(results, x)


# revision 14
# speedup vs baseline: 1.0322x; 1.0322x over previous
"""AttentionBlock (GroupNorm + single-head-dim-64 x4-head attention + proj,
residual) on 8 Trainium2 NeuronCores.

Sharding: data-parallel over batch (B=4) x query-position halves (2 per
batch) -> 8 shards, one per core. Each core:
  - GroupNorm(32) of its batch's x [256, 4096] (duplicated across the 2
    cores of a batch -- cheap),
  - qkv projection (k, v for all 4 heads over all 4096 positions; q only for
    the core's 2048 query positions),
  - flash-style attention: per (head, 512-query block), stream 128-wide key
    chunks: scores^T = k_chunk.T @ q -> exp on ScalarE -> accumulate
    exp(w).T @ [v | 1] on TensorE (the appended ones-column yields the
    softmax denominator for free),
  - divide by denominator, output projection, +bias +residual, DMA out.
Host gathers the 8 [256, 2048] outputs into [4, 256, 64, 64].

Query-half selection is done by ROTATING x's spatial axis on the host for
odd cores (softmax/groupnorm sums are permutation-invariant), so a single
SPMD program serves all 8 cores.
"""

import numpy as np

import concourse.bass as bass
import concourse.tile as tile
from concourse import bacc, mybir
from concourse.bass import AP

F32 = mybir.dt.float32
F32R = mybir.dt.float32r
AF = mybir.ActivationFunctionType
ALU = mybir.AluOpType

B, C, HH, WW = 4, 256, 64, 64
N = HH * WW            # 4096 spatial positions
NH = 4                 # heads
HD = C // NH           # 64 head dim
NG = 32                # groupnorm groups
EPS = 1e-5
NCORES = 8
THALF = N // 2         # 2048 query positions per core
TB = 512               # query-block width
NTB = THALF // TB      # 4
SCW = 128              # key-chunk width
NSC = N // SCW         # 32
SCALE = float(HD) ** -0.25

_PROGRAMS = {}         # (qkv_bias, proj_bias) -> compiled Bacc
_RUNNERS = {}          # id(nc) -> callable(in_maps) -> list of out dicts


def _build_program(with_qkv_bias: bool, with_proj_bias: bool, dbg: bool = False):
    nc = bacc.Bacc(
        "TRN2", target_bir_lowering=False, debug=False, num_devices=NCORES
    )
    if dbg:
        dxn = nc.declare_dram_parameter("dxn", [128, 2, N], F32R, isOutput=True)
        dk = nc.declare_dram_parameter("dk", [128, 2, N], F32R, isOutput=True)
        dq = nc.declare_dram_parameter("dq", [128, 2, THALF], F32R, isOutput=True)
        dvt = nc.declare_dram_parameter(
            "dvt", [128, NSC, NH, HD + 1], F32R, isOutput=True
        )
        dqk = nc.declare_dram_parameter("dqk", [128, 2 * TB], F32, isOutput=True)
        dacc = nc.declare_dram_parameter("dacc", [HD + 1, TB], F32, isOutput=True)
        dpc = nc.declare_dram_parameter("dpc", [128, 2, 2], F32, isOutput=True)
        dht = nc.declare_dram_parameter("dht", [128, 2, TB], F32R, isOutput=True)
        drec = nc.declare_dram_parameter("drec", [1, TB], F32, isOutput=True)
        dbc = nc.declare_dram_parameter("dbc", [64, TB], F32, isOutput=True)

    xb = nc.declare_dram_parameter("xb", [C, N], F32, isOutput=False)
    # columns: [0:256] q^T (scaled), [256:512] k^T (scaled), [512:768] v^T,
    # all in head-major order (col 64h+j of a block = qkv_w row 192h + off + j)
    wt = nc.declare_dram_parameter("wt", [C, 3 * C], F32R, isOutput=False)
    pt = nc.declare_dram_parameter("pt", [C, C], F32R, isOutput=False)  # proj_w.T
    gmat = nc.declare_dram_parameter("gmat", [128, 16], F32, isOutput=False)
    gmatt = nc.declare_dram_parameter("gmatt", [16, 128], F32, isOutput=False)
    gnw = nc.declare_dram_parameter("gnw", [C, 1], F32, isOutput=False)
    gnb = nc.declare_dram_parameter("gnb", [C, 1], F32, isOutput=False)
    if with_qkv_bias:
        bq = nc.declare_dram_parameter("bq", [C, 1], F32, isOutput=False)
        bk = nc.declare_dram_parameter("bk", [C, 1], F32, isOutput=False)
        bv = nc.declare_dram_parameter("bv", [1, C], F32, isOutput=False)
    if with_proj_bias:
        pb = nc.declare_dram_parameter("pb", [C, 1], F32, isOutput=False)
    yout = nc.declare_dram_parameter("y", [C, THALF], F32, isOutput=True)

    with tile.TileContext(nc) as tc:
        with tc.tile_pool(name="persist", bufs=1) as PP:
            x0 = PP.tile([128, N], F32)
            x1 = PP.tile([128, N], F32)
            nc.sync.dma_start(x0[:], xb[0:128, :])
            nc.sync.dma_start(x1[:], xb[128:256, :])
            xc = [x0, x1]

            wts = PP.tile([128, 2, 3 * C], F32R)
            nc.sync.dma_start(wts[:], wt.rearrange("(k p) m -> p k m", k=2))
            pts = PP.tile([128, 2, C], F32R)
            nc.sync.dma_start(pts[:], pt.rearrange("(k p) m -> p k m", k=2))
            gm = PP.tile([128, 16], F32)
            nc.sync.dma_start(gm[:], gmat[:])
            gmt = PP.tile([16, 128], F32)
            nc.sync.dma_start(gmt[:], gmatt[:])
            gns = PP.tile([128, 2, 2], F32)
            nc.sync.dma_start(gns[:, :, 0:1], gnw.rearrange("(k p) o -> p k o", k=2))
            nc.sync.dma_start(gns[:, :, 1:2], gnb.rearrange("(k p) o -> p k o", k=2))
            if with_qkv_bias:
                bqs = PP.tile([128, 2, 1], F32)
                nc.sync.dma_start(bqs[:], bq.rearrange("(k p) o -> p k o", k=2))
                bks = PP.tile([128, 2, 1], F32)
                nc.sync.dma_start(bks[:], bk.rearrange("(k p) o -> p k o", k=2))
                bvb = PP.tile([128, NH, HD], F32)
                bv_ap = AP(tensor=bv.tensor, offset=bv.offset,
                           ap=[[0, 128], [HD, NH], [1, HD]])
                nc.sync.dma_start(bvb[:], bv_ap)
            if with_proj_bias:
                pbs = PP.tile([128, 2, 1], F32)
                nc.sync.dma_start(pbs[:], pb.rearrange("(k p) o -> p k o", k=2))

            k_sb = PP.tile([128, 2, N], F32R)
            q_sb = PP.tile([128, 2, THALF], F32R)
            vTp = PP.tile([128, NSC, NH, HD + 1], F32R)
            nc.vector.memset(
                vTp[:, :, :, HD:HD + 1].bitcast(mybir.dt.uint32), 1065353216
            )  # 1.0f bit pattern (memset can't encode f32r)
            pcsb = PP.tile([128, 2, 2], F32)  # per-channel (scale, bias)
            eps16 = PP.tile([16, 1], F32)
            nc.vector.memset(eps16[:], EPS)

            # ---------------- GroupNorm statistics ----------------
            with (
                tc.tile_pool(name="gnp", bufs=2) as GP,
                tc.tile_pool(name="psg", bufs=1, space=bass.MemorySpace.PSUM) as PSG,
            ):
                psg = PSG.tile([16, 6], F32)
                for ci in range(2):
                    st = GP.tile([128, 8, 6], F32, tag="st")
                    for j in range(8):
                        nc.vector.bn_stats(
                            st[:, j, :], xc[ci][:, j * 512:(j + 1) * 512]
                        )
                    mv = GP.tile([128, 2], F32, tag="mv")
                    nc.vector.bn_aggr(mv[:], st[:])
                    s3 = GP.tile([128, 3], F32, tag="s3")
                    nc.vector.tensor_copy(s3[:, 0:2], mv[:])
                    nc.vector.tensor_mul(s3[:, 2:3], mv[:, 0:1], mv[:, 0:1])
                    # per-group sums over the 8 channels of each group
                    nc.tensor.matmul(
                        psg[:, 3 * ci:3 * ci + 3], gm[:], s3[:],
                        start=(ci == 0), stop=(ci == 1),
                    )
                gsb = GP.tile([16, 4], F32, tag="gsb")  # (rstd, mu*rstd) x2
                psgs = GP.tile([16, 6], F32, tag="psgs")
                nc.vector.tensor_copy(psgs[:], psg[:])
                for ci in range(2):
                    s1 = psgs[:, 3 * ci + 0:3 * ci + 1]
                    s2 = psgs[:, 3 * ci + 1:3 * ci + 2]
                    s3c = psgs[:, 3 * ci + 2:3 * ci + 3]
                    t4 = GP.tile([16, 4], F32, tag="t4")
                    nc.vector.tensor_scalar_mul(t4[:, 0:1], s1, 0.125)   # mu_g
                    nc.vector.tensor_add(t4[:, 1:2], s2, s3c)
                    nc.vector.tensor_scalar_mul(t4[:, 1:2], t4[:, 1:2], 0.125)
                    nc.vector.tensor_mul(t4[:, 2:3], t4[:, 0:1], t4[:, 0:1])
                    nc.vector.tensor_sub(t4[:, 1:2], t4[:, 1:2], t4[:, 2:3])
                    # sd = sqrt(var + eps)
                    nc.scalar.activation(
                        t4[:, 3:4], t4[:, 1:2], AF.Sqrt, bias=eps16[:], scale=1.0
                    )
                    nc.vector.reciprocal(gsb[:, 2 * ci:2 * ci + 1], t4[:, 3:4])
                    nc.vector.tensor_mul(
                        gsb[:, 2 * ci + 1:2 * ci + 2],
                        t4[:, 0:1], gsb[:, 2 * ci:2 * ci + 1],
                    )
                # group -> channel broadcast: G.T [16,128] expansion matmul
                g2cp = PSG.tile([128, 4], F32, tag="g2cp")
                nc.tensor.matmul(g2cp[:], gmt[:], gsb[:], start=True, stop=True)
                g2c = GP.tile([128, 4], F32, tag="g2c")
                nc.vector.tensor_copy(g2c[:], g2cp[:])
                for ci in range(2):
                    # scale_c = rstd*gn_w ; bias_c = gn_b - mu*rstd*gn_w
                    nc.vector.tensor_mul(
                        pcsb[:, ci, 0:1], g2c[:, 2 * ci:2 * ci + 1],
                        gns[:, ci, 0:1]
                    )
                    nc.vector.tensor_mul(
                        pcsb[:, ci, 1:2], g2c[:, 2 * ci + 1:2 * ci + 2],
                        gns[:, ci, 0:1]
                    )
                    nc.vector.tensor_sub(
                        pcsb[:, ci, 1:2], gns[:, ci, 1:2], pcsb[:, ci, 1:2]
                    )

            # ---------------- normalize + qkv projections ----------------
            with (
                tc.tile_pool(name="xnp", bufs=1) as XP,
                tc.tile_pool(name="psb", bufs=1, space=bass.MemorySpace.PSUM) as PSB,
            ):
                xn0 = XP.tile([128, N], F32R)
                xn1 = XP.tile([128, N], F32R)
                xn = [xn0, xn1]
                for ci in range(2):
                    nc.vector.tensor_scalar(
                        xn[ci][:], xc[ci][:],
                        pcsb[:, ci, 0:1], pcsb[:, ci, 1:2],
                        op0=ALU.mult, op1=ALU.add,
                    )
                if dbg:
                    nc.sync.dma_start(dxn[:, 0, :], xn0[:])
                    nc.sync.dma_start(dxn[:, 1, :], xn1[:])
                    nc.sync.dma_start(dpc[:], pcsb[:])
                # k (full n) and q (first THALF columns)
                for m in range(2):
                    for j in range(8):
                        ps = PSB.tile([128, 512], F32, tag="kq", bufs=3)
                        for kc in range(2):
                            nc.tensor.matmul(
                                ps[:],
                                wts[:, kc, 256 + 128 * m:256 + 128 * m + 128],
                                xn[kc][:, 512 * j:512 * j + 512],
                                start=(kc == 0), stop=(kc == 1),
                            )
                        if with_qkv_bias:
                            nc.scalar.activation(
                                k_sb[:, m, 512 * j:512 * j + 512], ps[:],
                                AF.Identity, bias=bks[:, m, :], scale=1.0,
                            )
                        else:
                            nc.vector.tensor_copy(
                                k_sb[:, m, 512 * j:512 * j + 512], ps[:]
                            )
                    for j in range(4):
                        ps = PSB.tile([128, 512], F32, tag="kq", bufs=3)
                        for kc in range(2):
                            nc.tensor.matmul(
                                ps[:],
                                wts[:, kc, 128 * m:128 * m + 128],
                                xn[kc][:, 512 * j:512 * j + 512],
                                start=(kc == 0), stop=(kc == 1),
                            )
                        if with_qkv_bias:
                            nc.scalar.activation(
                                q_sb[:, m, 512 * j:512 * j + 512], ps[:],
                                AF.Identity, bias=bqs[:, m, :], scale=1.0,
                            )
                        else:
                            nc.vector.tensor_copy(
                                q_sb[:, m, 512 * j:512 * j + 512], ps[:]
                            )
                # v^T directly: vT[s, c] = sum_ch xn[ch, s] * Wv^T[ch, c]
                for sc in range(NSC):
                    ps = PSB.tile([128, 256], F32, tag="v", bufs=3)
                    for kc in range(2):
                        nc.tensor.matmul(
                            ps[:],
                            xn[kc][:, SCW * sc:SCW * sc + SCW],
                            wts[:, kc, 512:768],
                            start=(kc == 0), stop=(kc == 1),
                        )
                    psr = ps[:].rearrange("p (h d) -> p h d", h=NH)
                    if with_qkv_bias:
                        nc.vector.tensor_add(vTp[:, sc, :, 0:HD], psr, bvb[:])
                    else:
                        nc.vector.tensor_copy(vTp[:, sc, :, 0:HD], psr)

            # ---------------- attention + output projection ----------------
            with (
                tc.tile_pool(name="att_sb", bufs=1) as AS,
                tc.tile_pool(name="att_dr", bufs=1,
                             space=bass.MemorySpace.DRAM) as DP,
                tc.tile_pool(name="ps_qk", bufs=1, space=bass.MemorySpace.PSUM) as PQK,
                tc.tile_pool(name="ps_acc", bufs=1, space=bass.MemorySpace.PSUM) as PAC,
                tc.tile_pool(name="ps_y", bufs=1, space=bass.MemorySpace.PSUM) as PY,
            ):
                if dbg:
                    nc.sync.dma_start(dk[:], k_sb[:])
                    nc.sync.dma_start(dq[:], q_sb[:])
                    nc.sync.dma_start(dvt[:], vTp[:])
                for tb in range(NTB):
                    ht = AS.tile([128, 2, TB], F32R, tag="ht", bufs=2)
                    for hp in range(2):
                        a0 = PAC.tile([HD + 1, TB], F32, tag="a0", bufs=1)
                        a1 = PAC.tile([HD + 1, TB], F32, tag="a1", bufs=1)
                        accs = [a0, a1]
                        for sc in range(NSC):
                            qk = PQK.tile([128, 2 * TB], F32, tag="qk", bufs=2)
                            for li in range(2):
                                nc.tensor.matmul(
                                    qk[:, TB * li:TB * li + TB],
                                    k_sb[64 * li:64 * li + 64, hp,
                                         SCW * sc:SCW * sc + SCW],
                                    q_sb[64 * li:64 * li + 64, hp,
                                         TB * tb:TB * tb + TB],
                                    start=True, stop=True,
                                )
                            ew = AS.tile([128, 2 * TB], F32R, tag="ew", bufs=3)
                            if dbg and tb == 0 and hp == 0 and sc == 0:
                                dqt = AS.tile([128, 2 * TB], F32, tag="dqt")
                                nc.vector.tensor_copy(dqt[:], qk[:])
                                nc.sync.dma_start(dqk[:], dqt[:])
                            nc.scalar.activation(ew[:], qk[:], AF.Exp)
                            for li in range(2):
                                nc.tensor.matmul(
                                    accs[li][:],
                                    vTp[:, sc, 2 * hp + li, :],
                                    ew[:, TB * li:TB * li + TB],
                                    start=(sc == 0), stop=(sc == NSC - 1),
                                )
                        if dbg and tb == 0 and hp == 0:
                            dat = AS.tile([HD + 1, TB], F32, tag="dat")
                            nc.vector.tensor_copy(dat[:], a0[:])
                            nc.sync.dma_start(dacc[:], dat[:])
                        for li in range(2):
                            acc = accs[li]
                            # reciprocal of the denominator row, partition-
                            # aligned at partition 64 (DVE lanes cannot move
                            # data across partitions)
                            rec = AS.tile([HD + 1, TB], F32, tag="rec", bufs=2)
                            nc.vector.reciprocal(
                                rec[HD:HD + 1, :], acc[HD:HD + 1, :]
                            )
                            # broadcast across partitions via DRAM bounce
                            rscr = DP.tile([1, TB], F32, tag="rscr", bufs=2)
                            nc.sync.dma_start(rscr[:], rec[HD:HD + 1, :])
                            bc = AS.tile([64, TB], F32, tag="bc", bufs=2)
                            rs = rscr[:]
                            nc.sync.dma_start(
                                bc[:],
                                AP(tensor=rs.tensor, offset=rs.offset,
                                   ap=[[0, 64], [1, TB]]),
                            )
                            if dbg and tb == 0 and hp == 0 and li == 0:
                                nc.sync.dma_start(drec[:], rec[HD:HD + 1, :])
                                nc.sync.dma_start(dbc[:], bc[:])
                            nc.vector.tensor_mul(
                                ht[64 * li:64 * li + 64, hp, :],
                                acc[0:HD, :], bc[:],
                            )
                    if dbg and tb == 0:
                        nc.sync.dma_start(dht[:], ht[:])
                    for m in range(2):
                        yp = PY.tile([128, TB], F32, tag="yp", bufs=2)
                        for kc in range(2):
                            nc.tensor.matmul(
                                yp[:],
                                pts[:, kc, 128 * m:128 * m + 128],
                                ht[:, kc, :],
                                start=(kc == 0), stop=(kc == 1),
                            )
                        yo = AS.tile([128, TB], F32, tag="yo", bufs=2)
                        nc.vector.tensor_add(
                            yo[:], yp[:], xc[m][:, TB * tb:TB * tb + TB]
                        )
                        if with_proj_bias:
                            nc.vector.tensor_scalar_add(
                                yo[:], yo[:], pbs[:, m, :]
                            )
                        nc.sync.dma_start(
                            yout[128 * m:128 * m + 128, TB * tb:TB * tb + TB],
                            yo[:],
                        )

    nc.compile()
    return nc


def get_program(with_qkv_bias: bool, with_proj_bias: bool):
    key = (with_qkv_bias, with_proj_bias)
    if key not in _PROGRAMS:
        _PROGRAMS[key] = _build_program(*key)
    return _PROGRAMS[key]


def _make_runner(nc):
    """Cached jit-compiled SPMD runner (mirrors bass2jax.run_bass_via_pjrt's
    multi-core branch but reusable across calls)."""
    import jax
    from jax.experimental.shard_map import shard_map
    from jax.sharding import Mesh, PartitionSpec
    from concourse import bass2jax

    bass2jax.install_neuronx_cc_hook()

    in_names, out_names, out_avals, zero_outs = [], [], [], []
    for alloc in nc.m.functions[0].allocations:
        if not isinstance(alloc, mybir.MemoryLocationSet):
            continue
        name = alloc.memorylocations[0].name
        if alloc.kind == "ExternalInput":
            in_names.append(name)
        elif alloc.kind == "ExternalOutput":
            out_names.append(name)
            shape = tuple(alloc.tensor_shape)
            dtype = mybir.dt.np(alloc.dtype)
            out_avals.append(jax.core.ShapedArray(shape, dtype))
            zero_outs.append(np.zeros(shape, dtype))
    n_params = len(in_names)
    n_outs = len(out_avals)
    all_names = in_names + out_names
    donate = tuple(range(n_params, n_params + n_outs))

    def _body(*args):
        outs = bass2jax._bass_exec_p.bind(
            *args,
            out_avals=tuple(out_avals),
            in_names=tuple(all_names),
            out_names=tuple(out_names),
            lowering_input_output_aliases=(),
            sim_require_finite=True,
            sim_require_nnan=True,
            nc=nc,
        )
        return tuple(outs)

    devices = jax.devices()[:NCORES]
    mesh = Mesh(np.asarray(devices), ("core",))
    in_specs = (PartitionSpec("core"),) * (n_params + n_outs)
    out_specs = (PartitionSpec("core"),) * n_outs
    sharded = jax.jit(
        shard_map(_body, mesh=mesh, in_specs=in_specs, out_specs=out_specs,
                  check_rep=False),
        donate_argnums=donate,
        keep_unused=True,
    )

    pid_name = nc.partition_id_tensor.name if nc.partition_id_tensor else None

    def run(in_maps):
        concat_in = [
            np.concatenate(
                [
                    np.asarray(m[name])
                    if name in m
                    else np.array([[c]], dtype=np.uint32)
                    for c, m in enumerate(in_maps)
                ],
                axis=0,
            )
            if name == pid_name
            else np.concatenate([np.asarray(m[name]) for m in in_maps], axis=0)
            for name in in_names
        ]
        concat_zero = [
            np.zeros((NCORES * z.shape[0], *z.shape[1:]), z.dtype)
            for z in zero_outs
        ]
        out_arrs = sharded(*concat_in, *concat_zero)
        return [
            {
                name: np.asarray(out_arrs[i]).reshape(
                    NCORES, *out_avals[i].shape
                )[c]
                for i, name in enumerate(out_names)
            }
            for c in range(NCORES)
        ]

    run.in_names = in_names
    run.out_names = out_names
    return run


def get_runner(nc):
    key = id(nc)
    if key not in _RUNNERS:
        _RUNNERS[key] = _make_runner(nc)
    return _RUNNERS[key]


def prep_host_inputs(x, gn_w, gn_b, qkv_w, qkv_b, proj_w, proj_b):
    """Host-side weight layout transforms + per-core input maps."""
    x = np.asarray(x, np.float32)
    gn_w = np.asarray(gn_w, np.float32)
    gn_b = np.asarray(gn_b, np.float32)
    qkv_w = np.asarray(qkv_w, np.float32)
    qkv_b = np.asarray(qkv_b, np.float32)
    proj_w = np.asarray(proj_w, np.float32)
    proj_b = np.asarray(proj_b, np.float32)

    # head-major row indices into qkv_w for q / k / v
    hidx = np.arange(NH)[:, None] * (3 * HD) + np.arange(HD)[None, :]
    qrows = hidx.reshape(-1)
    krows = (hidx + HD).reshape(-1)
    vrows = (hidx + 2 * HD).reshape(-1)
    wt = np.empty((C, 3 * C), np.float32)
    wt[:, 0:C] = qkv_w[qrows].T * SCALE
    wt[:, C:2 * C] = qkv_w[krows].T * SCALE
    wt[:, 2 * C:3 * C] = qkv_w[vrows].T
    pt = np.ascontiguousarray(proj_w.T)

    gmat = np.zeros((128, 16), np.float32)
    gmat[np.arange(128), np.arange(128) // 8] = 1.0
    gmatt = np.ascontiguousarray(gmat.T)

    with_qkv_bias = bool(np.any(qkv_b))
    with_proj_bias = bool(np.any(proj_b))

    shared = dict(
        wt=wt, pt=pt, gmat=gmat, gmatt=gmatt,
        gnw=gn_w.reshape(C, 1).astype(np.float32),
        gnb=gn_b.reshape(C, 1).astype(np.float32),
    )
    if with_qkv_bias:
        shared["bq"] = (qkv_b[qrows] * SCALE).reshape(C, 1).astype(np.float32)
        shared["bk"] = (qkv_b[krows] * SCALE).reshape(C, 1).astype(np.float32)
        shared["bv"] = qkv_b[vrows].reshape(1, C).astype(np.float32)
    if with_proj_bias:
        shared["pb"] = proj_b.reshape(C, 1).astype(np.float32)

    xf = x.reshape(B, C, N)
    in_maps = []
    for core in range(NCORES):
        b, half = core // 2, core % 2
        if half == 0:
            xbc = np.ascontiguousarray(xf[b])
        else:
            xbc = np.ascontiguousarray(
                np.concatenate([xf[b][:, THALF:], xf[b][:, :THALF]], axis=1)
            )
        m = dict(shared)
        m["xb"] = xbc
        in_maps.append(m)
    return in_maps, with_qkv_bias, with_proj_bias


def assemble_output(results, x):
    out = np.empty((B, C, N), np.float32)
    for core in range(NCORES):
        b, half = core // 2, core % 2
        out[b][:, half * THALF:(half + 1) * THALF] = results[core]["y"]
    return out.reshape(B, C, HH, WW)


def kernel(x, gn_w, gn_b, qkv_w, qkv_b, proj_w, proj_b):
    in_maps, qb, pbf = prep_host_inputs(
        x, gn_w, gn_b, qkv_w, qkv_b, proj_w, proj_b
    )
    nc = get_program(qb, pbf)
    run = get_runner(nc)
    results = run(in_maps)
    return assemble_output(results, x)


# revision 15
# speedup vs baseline: 1.0375x; 1.0051x over previous
"""AttentionBlock (GroupNorm + single-head-dim-64 x4-head attention + proj,
residual) on 8 Trainium2 NeuronCores.

Sharding: data-parallel over batch (B=4) x query-position halves (2 per
batch) -> 8 shards, one per core. Each core:
  - GroupNorm(32) of its batch's x [256, 4096] (duplicated across the 2
    cores of a batch -- cheap),
  - qkv projection (k, v for all 4 heads over all 4096 positions; q only for
    the core's 2048 query positions),
  - flash-style attention: per (head, 512-query block), stream 128-wide key
    chunks: scores^T = k_chunk.T @ q -> exp on ScalarE -> accumulate
    exp(w).T @ [v | 1] on TensorE (the appended ones-column yields the
    softmax denominator for free),
  - divide by denominator, output projection, +bias +residual, DMA out.
Host gathers the 8 [256, 2048] outputs into [4, 256, 64, 64].

Query-half selection is done by ROTATING x's spatial axis on the host for
odd cores (softmax/groupnorm sums are permutation-invariant), so a single
SPMD program serves all 8 cores.
"""

import numpy as np

import concourse.bass as bass
import concourse.tile as tile
from concourse import bacc, mybir
from concourse.bass import AP

F32 = mybir.dt.float32
F32R = mybir.dt.float32r
AF = mybir.ActivationFunctionType
ALU = mybir.AluOpType

B, C, HH, WW = 4, 256, 64, 64
N = HH * WW            # 4096 spatial positions
NH = 4                 # heads
HD = C // NH           # 64 head dim
NG = 32                # groupnorm groups
EPS = 1e-5
NCORES = 8
THALF = N // 2         # 2048 query positions per core
TB = 512               # query-block width
NTB = THALF // TB      # 4
SCW = 128              # key-chunk width
NSC = N // SCW         # 32
SCALE = float(HD) ** -0.25

_PROGRAMS = {}         # (qkv_bias, proj_bias) -> compiled Bacc
_RUNNERS = {}          # id(nc) -> callable(in_maps) -> list of out dicts

# ---- custom DVE exp: exp(x) = (c0 + c1*u + c2*u^2)^8 with u = x/8 -------
# (scores are pre-scaled by 1/8 on the host; the ScalarE path compensates
# with activation scale=8.0). 8 ALU stages exactly; max rel err ~1.1e-3 on
# scores in [-1.2, 1.2], which lands ~2e-6 in the final output after the
# softmax normalization and the small attention/residual ratio.
EXPC = (1.0000157, 1.0028037, 0.49929515)


def _register_exp8():
    from concourse import dve_ops
    from concourse.dve_spec import C0, C1, C2, Spec, Src0, sq

    if any(o.name == "EXP8_APPROX_ANT" for o in dve_ops.OPS):
        return next(o for o in dve_ops.OPS if o.name == "EXP8_APPROX_ANT")

    u = Src0
    q = C0 + u * C1 + sq(u) * C2
    body = sq(sq(sq(q)))

    def ref(in0, in1, c0, c1, c2):
        qq = c0 + in0 * c1 + in0 * in0 * c2
        qq = qq * qq
        qq = qq * qq
        qq = qq * qq
        return qq.astype(np.float32)

    op = dve_ops.DveOp(
        "EXP8_APPROX_ANT",
        Spec(body=body, reference=ref),
        subdim=False,
        uops_sha={"v3": "5382d78ba1590096", "v4": "cbef02bfe377eaa0"},
    )
    dve_ops.OPS.append(op)
    dve_ops._SUB_OPCODE_FOR_NAME[op.name] = (
        max(dve_ops._SUB_OPCODE_FOR_NAME.values()) + 1
    )
    dve_ops.CUSTOM_DVE_SPECS[op.name] = op.spec
    return op


EXP8_OP = _register_exp8()
# of every 3 key-chunk iterations, this many run exp on ScalarE (rest on DVE)
ACT_EXP_PATTERN = (True, True, False)


def _build_program(with_qkv_bias: bool, with_proj_bias: bool, dbg: bool = False):
    nc = bacc.Bacc(
        "TRN2", target_bir_lowering=False, debug=False, num_devices=NCORES
    )
    if dbg:
        dxn = nc.declare_dram_parameter("dxn", [128, 2, N], F32R, isOutput=True)
        dk = nc.declare_dram_parameter("dk", [128, 2, N], F32R, isOutput=True)
        dq = nc.declare_dram_parameter("dq", [128, 2, THALF], F32R, isOutput=True)
        dvt = nc.declare_dram_parameter(
            "dvt", [128, NSC, NH, HD + 1], F32R, isOutput=True
        )
        dqk = nc.declare_dram_parameter("dqk", [128, 2 * TB], F32, isOutput=True)
        dacc = nc.declare_dram_parameter("dacc", [HD + 1, TB], F32, isOutput=True)
        dpc = nc.declare_dram_parameter("dpc", [128, 2, 2], F32, isOutput=True)
        dht = nc.declare_dram_parameter("dht", [128, 2, TB], F32R, isOutput=True)
        drec = nc.declare_dram_parameter("drec", [1, TB], F32, isOutput=True)
        dbc = nc.declare_dram_parameter("dbc", [64, TB], F32, isOutput=True)

    xb = nc.declare_dram_parameter("xb", [C, N], F32, isOutput=False)
    # columns: [0:256] q^T (scaled), [256:512] k^T (scaled), [512:768] v^T,
    # all in head-major order (col 64h+j of a block = qkv_w row 192h + off + j)
    wt = nc.declare_dram_parameter("wt", [C, 3 * C], F32R, isOutput=False)
    pt = nc.declare_dram_parameter("pt", [C, C], F32R, isOutput=False)  # proj_w.T
    gmat = nc.declare_dram_parameter("gmat", [128, 16], F32, isOutput=False)
    gmatt = nc.declare_dram_parameter("gmatt", [16, 128], F32, isOutput=False)
    gnw = nc.declare_dram_parameter("gnw", [C, 1], F32, isOutput=False)
    gnb = nc.declare_dram_parameter("gnb", [C, 1], F32, isOutput=False)
    if with_qkv_bias:
        bq = nc.declare_dram_parameter("bq", [C, 1], F32, isOutput=False)
        bk = nc.declare_dram_parameter("bk", [C, 1], F32, isOutput=False)
        bv = nc.declare_dram_parameter("bv", [1, C], F32, isOutput=False)
    if with_proj_bias:
        pb = nc.declare_dram_parameter("pb", [C, 1], F32, isOutput=False)
    yout = nc.declare_dram_parameter("y", [C, THALF], F32, isOutput=True)

    with tile.TileContext(nc) as tc:
        with tc.tile_pool(name="persist", bufs=1) as PP:
            x0 = PP.tile([128, N], F32)
            x1 = PP.tile([128, N], F32)
            nc.sync.dma_start(x0[:], xb[0:128, :])
            nc.sync.dma_start(x1[:], xb[128:256, :])
            xc = [x0, x1]

            wts = PP.tile([128, 2, 3 * C], F32R)
            nc.sync.dma_start(wts[:], wt.rearrange("(k p) m -> p k m", k=2))
            pts = PP.tile([128, 2, C], F32R)
            nc.sync.dma_start(pts[:], pt.rearrange("(k p) m -> p k m", k=2))
            gm = PP.tile([128, 16], F32)
            nc.sync.dma_start(gm[:], gmat[:])
            gmt = PP.tile([16, 128], F32)
            nc.sync.dma_start(gmt[:], gmatt[:])
            gns = PP.tile([128, 2, 2], F32)
            nc.sync.dma_start(gns[:, :, 0:1], gnw.rearrange("(k p) o -> p k o", k=2))
            nc.sync.dma_start(gns[:, :, 1:2], gnb.rearrange("(k p) o -> p k o", k=2))
            if with_qkv_bias:
                bqs = PP.tile([128, 2, 1], F32)
                nc.sync.dma_start(bqs[:], bq.rearrange("(k p) o -> p k o", k=2))
                bks = PP.tile([128, 2, 1], F32)
                nc.sync.dma_start(bks[:], bk.rearrange("(k p) o -> p k o", k=2))
                bvb = PP.tile([128, NH, HD], F32)
                bv_ap = AP(tensor=bv.tensor, offset=bv.offset,
                           ap=[[0, 128], [HD, NH], [1, HD]])
                nc.sync.dma_start(bvb[:], bv_ap)
            if with_proj_bias:
                pbs = PP.tile([128, 2, 1], F32)
                nc.sync.dma_start(pbs[:], pb.rearrange("(k p) o -> p k o", k=2))

            k_sb = PP.tile([128, 2, N], F32R)
            q_sb = PP.tile([128, 2, THALF], F32R)
            vTp = PP.tile([128, NSC, NH, HD + 1], F32R)
            nc.vector.memset(
                vTp[:, :, :, HD:HD + 1].bitcast(mybir.dt.uint32), 1065353216
            )  # 1.0f bit pattern (memset can't encode f32r)
            pcsb = PP.tile([128, 2, 2], F32)  # per-channel (scale, bias)
            eps16 = PP.tile([16, 1], F32)
            nc.vector.memset(eps16[:], EPS)

            # ---------------- GroupNorm statistics ----------------
            with (
                tc.tile_pool(name="gnp", bufs=2) as GP,
                tc.tile_pool(name="psg", bufs=1, space=bass.MemorySpace.PSUM) as PSG,
            ):
                psg = PSG.tile([16, 6], F32)
                for ci in range(2):
                    st = GP.tile([128, 8, 6], F32, tag="st")
                    for j in range(8):
                        nc.vector.bn_stats(
                            st[:, j, :], xc[ci][:, j * 512:(j + 1) * 512]
                        )
                    mv = GP.tile([128, 2], F32, tag="mv")
                    nc.vector.bn_aggr(mv[:], st[:])
                    s3 = GP.tile([128, 3], F32, tag="s3")
                    nc.vector.tensor_copy(s3[:, 0:2], mv[:])
                    nc.vector.tensor_mul(s3[:, 2:3], mv[:, 0:1], mv[:, 0:1])
                    # per-group sums over the 8 channels of each group
                    nc.tensor.matmul(
                        psg[:, 3 * ci:3 * ci + 3], gm[:], s3[:],
                        start=(ci == 0), stop=(ci == 1),
                    )
                gsb = GP.tile([16, 4], F32, tag="gsb")  # (rstd, mu*rstd) x2
                psgs = GP.tile([16, 6], F32, tag="psgs")
                nc.vector.tensor_copy(psgs[:], psg[:])
                for ci in range(2):
                    s1 = psgs[:, 3 * ci + 0:3 * ci + 1]
                    s2 = psgs[:, 3 * ci + 1:3 * ci + 2]
                    s3c = psgs[:, 3 * ci + 2:3 * ci + 3]
                    t4 = GP.tile([16, 4], F32, tag="t4")
                    nc.vector.tensor_scalar_mul(t4[:, 0:1], s1, 0.125)   # mu_g
                    nc.vector.tensor_add(t4[:, 1:2], s2, s3c)
                    nc.vector.tensor_scalar_mul(t4[:, 1:2], t4[:, 1:2], 0.125)
                    nc.vector.tensor_mul(t4[:, 2:3], t4[:, 0:1], t4[:, 0:1])
                    nc.vector.tensor_sub(t4[:, 1:2], t4[:, 1:2], t4[:, 2:3])
                    # sd = sqrt(var + eps)
                    nc.scalar.activation(
                        t4[:, 3:4], t4[:, 1:2], AF.Sqrt, bias=eps16[:], scale=1.0
                    )
                    nc.vector.reciprocal(gsb[:, 2 * ci:2 * ci + 1], t4[:, 3:4])
                    nc.vector.tensor_mul(
                        gsb[:, 2 * ci + 1:2 * ci + 2],
                        t4[:, 0:1], gsb[:, 2 * ci:2 * ci + 1],
                    )
                # group -> channel broadcast: G.T [16,128] expansion matmul
                g2cp = PSG.tile([128, 4], F32, tag="g2cp")
                nc.tensor.matmul(g2cp[:], gmt[:], gsb[:], start=True, stop=True)
                g2c = GP.tile([128, 4], F32, tag="g2c")
                nc.vector.tensor_copy(g2c[:], g2cp[:])
                for ci in range(2):
                    # scale_c = rstd*gn_w ; bias_c = gn_b - mu*rstd*gn_w
                    nc.vector.tensor_mul(
                        pcsb[:, ci, 0:1], g2c[:, 2 * ci:2 * ci + 1],
                        gns[:, ci, 0:1]
                    )
                    nc.vector.tensor_mul(
                        pcsb[:, ci, 1:2], g2c[:, 2 * ci + 1:2 * ci + 2],
                        gns[:, ci, 0:1]
                    )
                    nc.vector.tensor_sub(
                        pcsb[:, ci, 1:2], gns[:, ci, 1:2], pcsb[:, ci, 1:2]
                    )

            # ---------------- normalize + qkv projections ----------------
            with (
                tc.tile_pool(name="xnp", bufs=1) as XP,
                tc.tile_pool(name="psb", bufs=1, space=bass.MemorySpace.PSUM) as PSB,
            ):
                xn0 = XP.tile([128, N], F32R)
                xn1 = XP.tile([128, N], F32R)
                xn = [xn0, xn1]
                for ci in range(2):
                    nc.vector.tensor_scalar(
                        xn[ci][:], xc[ci][:],
                        pcsb[:, ci, 0:1], pcsb[:, ci, 1:2],
                        op0=ALU.mult, op1=ALU.add,
                    )
                if dbg:
                    nc.sync.dma_start(dxn[:, 0, :], xn0[:])
                    nc.sync.dma_start(dxn[:, 1, :], xn1[:])
                    nc.sync.dma_start(dpc[:], pcsb[:])
                # k (full n) and q (first THALF columns)
                for m in range(2):
                    for j in range(8):
                        ps = PSB.tile([128, 512], F32, tag="kq", bufs=3)
                        for kc in range(2):
                            nc.tensor.matmul(
                                ps[:],
                                wts[:, kc, 256 + 128 * m:256 + 128 * m + 128],
                                xn[kc][:, 512 * j:512 * j + 512],
                                start=(kc == 0), stop=(kc == 1),
                            )
                        if with_qkv_bias:
                            nc.scalar.activation(
                                k_sb[:, m, 512 * j:512 * j + 512], ps[:],
                                AF.Identity, bias=bks[:, m, :], scale=1.0,
                            )
                        else:
                            nc.vector.tensor_copy(
                                k_sb[:, m, 512 * j:512 * j + 512], ps[:]
                            )
                    for j in range(4):
                        ps = PSB.tile([128, 512], F32, tag="kq", bufs=3)
                        for kc in range(2):
                            nc.tensor.matmul(
                                ps[:],
                                wts[:, kc, 128 * m:128 * m + 128],
                                xn[kc][:, 512 * j:512 * j + 512],
                                start=(kc == 0), stop=(kc == 1),
                            )
                        if with_qkv_bias:
                            nc.scalar.activation(
                                q_sb[:, m, 512 * j:512 * j + 512], ps[:],
                                AF.Identity, bias=bqs[:, m, :], scale=1.0,
                            )
                        else:
                            nc.vector.tensor_copy(
                                q_sb[:, m, 512 * j:512 * j + 512], ps[:]
                            )
                # v^T directly: vT[s, c] = sum_ch xn[ch, s] * Wv^T[ch, c]
                for sc in range(NSC):
                    ps = PSB.tile([128, 256], F32, tag="v", bufs=3)
                    for kc in range(2):
                        nc.tensor.matmul(
                            ps[:],
                            xn[kc][:, SCW * sc:SCW * sc + SCW],
                            wts[:, kc, 512:768],
                            start=(kc == 0), stop=(kc == 1),
                        )
                    psr = ps[:].rearrange("p (h d) -> p h d", h=NH)
                    if with_qkv_bias:
                        nc.vector.tensor_add(vTp[:, sc, :, 0:HD], psr, bvb[:])
                    else:
                        nc.vector.tensor_copy(vTp[:, sc, :, 0:HD], psr)

            # ---------------- attention + output projection ----------------
            with (
                tc.tile_pool(name="att_sb", bufs=1) as AS,
                tc.tile_pool(name="att_dr", bufs=1,
                             space=bass.MemorySpace.DRAM) as DP,
                tc.tile_pool(name="ps_qk", bufs=1, space=bass.MemorySpace.PSUM) as PQK,
                tc.tile_pool(name="ps_acc", bufs=1, space=bass.MemorySpace.PSUM) as PAC,
                tc.tile_pool(name="ps_y", bufs=1, space=bass.MemorySpace.PSUM) as PY,
            ):
                if dbg:
                    nc.sync.dma_start(dk[:], k_sb[:])
                    nc.sync.dma_start(dq[:], q_sb[:])
                    nc.sync.dma_start(dvt[:], vTp[:])
                for tb in range(NTB):
                    ht = AS.tile([128, 2, TB], F32R, tag="ht", bufs=2)
                    for hp in range(2):
                        a0 = PAC.tile([HD + 1, TB], F32, tag="a0", bufs=1)
                        a1 = PAC.tile([HD + 1, TB], F32, tag="a1", bufs=1)
                        accs = [a0, a1]
                        for sc in range(NSC):
                            qk = PQK.tile([128, 2 * TB], F32, tag="qk", bufs=2)
                            for li in range(2):
                                nc.tensor.matmul(
                                    qk[:, TB * li:TB * li + TB],
                                    k_sb[64 * li:64 * li + 64, hp,
                                         SCW * sc:SCW * sc + SCW],
                                    q_sb[64 * li:64 * li + 64, hp,
                                         TB * tb:TB * tb + TB],
                                    start=True, stop=True,
                                )
                            ew = AS.tile([128, 2 * TB], F32R, tag="ew", bufs=3)
                            if dbg and tb == 0 and hp == 0 and sc == 0:
                                dqt = AS.tile([128, 2 * TB], F32, tag="dqt")
                                nc.vector.tensor_copy(dqt[:], qk[:])
                                nc.sync.dma_start(dqk[:], dqt[:])
                            if ACT_EXP_PATTERN[sc % len(ACT_EXP_PATTERN)]:
                                nc.scalar.activation(
                                    ew[:], qk[:], AF.Exp, scale=8.0
                                )
                            else:
                                nc.vector._custom_dve(
                                    EXP8_OP, out=ew[:], in0=qk[:],
                                    s0=EXPC[0], s1=EXPC[1], imm2=EXPC[2],
                                )
                            for li in range(2):
                                nc.tensor.matmul(
                                    accs[li][:],
                                    vTp[:, sc, 2 * hp + li, :],
                                    ew[:, TB * li:TB * li + TB],
                                    start=(sc == 0), stop=(sc == NSC - 1),
                                )
                        if dbg and tb == 0 and hp == 0:
                            dat = AS.tile([HD + 1, TB], F32, tag="dat")
                            nc.vector.tensor_copy(dat[:], a0[:])
                            nc.sync.dma_start(dacc[:], dat[:])
                        for li in range(2):
                            acc = accs[li]
                            # reciprocal of the denominator row, partition-
                            # aligned at partition 64 (DVE lanes cannot move
                            # data across partitions)
                            rec = AS.tile([HD + 1, TB], F32, tag="rec", bufs=2)
                            nc.vector.reciprocal(
                                rec[HD:HD + 1, :], acc[HD:HD + 1, :]
                            )
                            # broadcast across partitions via DRAM bounce
                            rscr = DP.tile([1, TB], F32, tag="rscr", bufs=2)
                            nc.sync.dma_start(rscr[:], rec[HD:HD + 1, :])
                            bc = AS.tile([64, TB], F32, tag="bc", bufs=2)
                            rs = rscr[:]
                            nc.sync.dma_start(
                                bc[:],
                                AP(tensor=rs.tensor, offset=rs.offset,
                                   ap=[[0, 64], [1, TB]]),
                            )
                            if dbg and tb == 0 and hp == 0 and li == 0:
                                nc.sync.dma_start(drec[:], rec[HD:HD + 1, :])
                                nc.sync.dma_start(dbc[:], bc[:])
                            nc.vector.tensor_mul(
                                ht[64 * li:64 * li + 64, hp, :],
                                acc[0:HD, :], bc[:],
                            )
                    if dbg and tb == 0:
                        nc.sync.dma_start(dht[:], ht[:])
                    for m in range(2):
                        yp = PY.tile([128, TB], F32, tag="yp", bufs=2)
                        for kc in range(2):
                            nc.tensor.matmul(
                                yp[:],
                                pts[:, kc, 128 * m:128 * m + 128],
                                ht[:, kc, :],
                                start=(kc == 0), stop=(kc == 1),
                            )
                        yo = AS.tile([128, TB], F32, tag="yo", bufs=2)
                        nc.vector.tensor_add(
                            yo[:], yp[:], xc[m][:, TB * tb:TB * tb + TB]
                        )
                        if with_proj_bias:
                            nc.vector.tensor_scalar_add(
                                yo[:], yo[:], pbs[:, m, :]
                            )
                        nc.sync.dma_start(
                            yout[128 * m:128 * m + 128, TB * tb:TB * tb + TB],
                            yo[:],
                        )

    nc.compile()
    return nc


def get_program(with_qkv_bias: bool, with_proj_bias: bool):
    key = (with_qkv_bias, with_proj_bias)
    if key not in _PROGRAMS:
        _PROGRAMS[key] = _build_program(*key)
    return _PROGRAMS[key]


def _make_runner(nc):
    """Cached jit-compiled SPMD runner (mirrors bass2jax.run_bass_via_pjrt's
    multi-core branch but reusable across calls)."""
    import jax
    from jax.experimental.shard_map import shard_map
    from jax.sharding import Mesh, PartitionSpec
    from concourse import bass2jax

    bass2jax.install_neuronx_cc_hook()

    in_names, out_names, out_avals, zero_outs = [], [], [], []
    for alloc in nc.m.functions[0].allocations:
        if not isinstance(alloc, mybir.MemoryLocationSet):
            continue
        name = alloc.memorylocations[0].name
        if alloc.kind == "ExternalInput":
            in_names.append(name)
        elif alloc.kind == "ExternalOutput":
            out_names.append(name)
            shape = tuple(alloc.tensor_shape)
            dtype = mybir.dt.np(alloc.dtype)
            out_avals.append(jax.core.ShapedArray(shape, dtype))
            zero_outs.append(np.zeros(shape, dtype))
    n_params = len(in_names)
    n_outs = len(out_avals)
    all_names = in_names + out_names
    donate = tuple(range(n_params, n_params + n_outs))

    def _body(*args):
        outs = bass2jax._bass_exec_p.bind(
            *args,
            out_avals=tuple(out_avals),
            in_names=tuple(all_names),
            out_names=tuple(out_names),
            lowering_input_output_aliases=(),
            sim_require_finite=True,
            sim_require_nnan=True,
            nc=nc,
        )
        return tuple(outs)

    devices = jax.devices()[:NCORES]
    mesh = Mesh(np.asarray(devices), ("core",))
    in_specs = (PartitionSpec("core"),) * (n_params + n_outs)
    out_specs = (PartitionSpec("core"),) * n_outs
    sharded = jax.jit(
        shard_map(_body, mesh=mesh, in_specs=in_specs, out_specs=out_specs,
                  check_rep=False),
        donate_argnums=donate,
        keep_unused=True,
    )

    pid_name = nc.partition_id_tensor.name if nc.partition_id_tensor else None

    def run(in_maps):
        concat_in = [
            np.concatenate(
                [
                    np.asarray(m[name])
                    if name in m
                    else np.array([[c]], dtype=np.uint32)
                    for c, m in enumerate(in_maps)
                ],
                axis=0,
            )
            if name == pid_name
            else np.concatenate([np.asarray(m[name]) for m in in_maps], axis=0)
            for name in in_names
        ]
        concat_zero = [
            np.zeros((NCORES * z.shape[0], *z.shape[1:]), z.dtype)
            for z in zero_outs
        ]
        out_arrs = sharded(*concat_in, *concat_zero)
        return [
            {
                name: np.asarray(out_arrs[i]).reshape(
                    NCORES, *out_avals[i].shape
                )[c]
                for i, name in enumerate(out_names)
            }
            for c in range(NCORES)
        ]

    run.in_names = in_names
    run.out_names = out_names
    return run


def get_runner(nc):
    key = id(nc)
    if key not in _RUNNERS:
        _RUNNERS[key] = _make_runner(nc)
    return _RUNNERS[key]


def prep_host_inputs(x, gn_w, gn_b, qkv_w, qkv_b, proj_w, proj_b):
    """Host-side weight layout transforms + per-core input maps."""
    x = np.asarray(x, np.float32)
    gn_w = np.asarray(gn_w, np.float32)
    gn_b = np.asarray(gn_b, np.float32)
    qkv_w = np.asarray(qkv_w, np.float32)
    qkv_b = np.asarray(qkv_b, np.float32)
    proj_w = np.asarray(proj_w, np.float32)
    proj_b = np.asarray(proj_b, np.float32)

    # head-major row indices into qkv_w for q / k / v
    hidx = np.arange(NH)[:, None] * (3 * HD) + np.arange(HD)[None, :]
    qrows = hidx.reshape(-1)
    krows = (hidx + HD).reshape(-1)
    vrows = (hidx + 2 * HD).reshape(-1)
    wt = np.empty((C, 3 * C), np.float32)
    wt[:, 0:C] = qkv_w[qrows].T * (SCALE / 8.0)
    wt[:, C:2 * C] = qkv_w[krows].T * SCALE
    wt[:, 2 * C:3 * C] = qkv_w[vrows].T
    pt = np.ascontiguousarray(proj_w.T)

    gmat = np.zeros((128, 16), np.float32)
    gmat[np.arange(128), np.arange(128) // 8] = 1.0
    gmatt = np.ascontiguousarray(gmat.T)

    with_qkv_bias = bool(np.any(qkv_b))
    with_proj_bias = bool(np.any(proj_b))

    shared = dict(
        wt=wt, pt=pt, gmat=gmat, gmatt=gmatt,
        gnw=gn_w.reshape(C, 1).astype(np.float32),
        gnb=gn_b.reshape(C, 1).astype(np.float32),
    )
    if with_qkv_bias:
        shared["bq"] = (qkv_b[qrows] * (SCALE / 8.0)).reshape(C, 1).astype(np.float32)
        shared["bk"] = (qkv_b[krows] * SCALE).reshape(C, 1).astype(np.float32)
        shared["bv"] = qkv_b[vrows].reshape(1, C).astype(np.float32)
    if with_proj_bias:
        shared["pb"] = proj_b.reshape(C, 1).astype(np.float32)

    xf = x.reshape(B, C, N)
    in_maps = []
    for core in range(NCORES):
        b, half = core // 2, core % 2
        if half == 0:
            xbc = np.ascontiguousarray(xf[b])
        else:
            xbc = np.ascontiguousarray(
                np.concatenate([xf[b][:, THALF:], xf[b][:, :THALF]], axis=1)
            )
        m = dict(shared)
        m["xb"] = xbc
        in_maps.append(m)
    return in_maps, with_qkv_bias, with_proj_bias


def assemble_output(results, x):
    out = np.empty((B, C, N), np.float32)
    for core in range(NCORES):
        b, half = core // 2, core % 2
        out[b][:, half * THALF:(half + 1) * THALF] = results[core]["y"]
    return out.reshape(B, C, HH, WW)


def kernel(x, gn_w, gn_b, qkv_w, qkv_b, proj_w, proj_b):
    in_maps, qb, pbf = prep_host_inputs(
        x, gn_w, gn_b, qkv_w, qkv_b, proj_w, proj_b
    )
    nc = get_program(qb, pbf)
    run = get_runner(nc)
    results = run(in_maps)
    return assemble_output(results, x)
